# revision 32
# baseline (speedup 1.0000x reference)
"""MoE layer (top-2 of 24 experts, d_model=1024, d_ff=4096, T=4096 tokens)
on 8 Trainium2 NeuronCores.

Strategy (expert-parallel, host-routed, exact-capacity):
  - Host computes the gate, top-2 ids and softmax probs, sorts the routed
    (token, expert) pairs by expert, and deals experts 3 per core (slot j
    of core c = j-th-octile expert c) so slot capacities are the octile
    maxima of the raw counts -- EXACT, no 128 rounding.
  - Device kernel per slot expert (W = slot capacity, tokens are always
    the matmul free dim so W needs no alignment):
      phase A: hT[m] = gelu(w1[k,m].T @ xT[k] + b1[m])   [128 dff, W]
      phase B: yT[m] += w2[k,m].T @ hT[k]                [128 dm, W]
    yT is written transposed [d_model, tokens]; the top-2 softmax probs
    and the final combine (and optional b2 term) are applied on host.
  - Weights are re-laid-out on host so each DMA is one large contiguous
    block: w1 in 8 chunks of [128, 4096] (1 MB each), w2 in 4 chunks of
    [128, 8192] (2.1 MB each). w1 of expert e+1 prefetches during
    phase B of e; w2 of e loads during phase A of e (single-buffered).

Matmuls in bf16 with fp32 PSUM accumulation; b1 applied exactly as the
ACT per-partition bias.
"""

import numpy as np
import ml_dtypes

P = 128
D_MODEL = 1024
D_FF = 4096
NUM_EXPERTS = 24
TOP_K = 2
N_CORES = 8
E_LOC = NUM_EXPERTS // N_CORES   # 3 experts per core
KD = D_MODEL // P                # 8  k-chunks over d_model
KF = D_FF // P                   # 32 k-chunks over d_ff
MD = D_MODEL // P                # 8  m-tiles over d_model (phase B)
W1C = 16                         # w1 streamed in 16 col-chunks of 256
W1CW = D_FF // W1C               # 512 dff cols per chunk
MPC = W1CW // P                  # 4 m-tiles per w1 chunk
W2C = 4                          # w2 loaded in 4 chunks of 8 k-tiles
KPC = KF // W2C                  # 8 k-tiles per w2 chunk
BF16 = ml_dtypes.bfloat16


def _build(Cs):
    """Per-core Bass program (SPMD: same program, per-core data).

    Cs: per-slot token capacities (exact counts, no alignment needed).
    """
    import concourse.bacc as bacc
    import concourse.mybir as mybir
    from concourse.tile import TileContext

    dt = mybir.dt.bfloat16
    f32 = mybir.dt.float32
    CT = sum(Cs)
    offs = [sum(Cs[:j]) for j in range(E_LOC)]

    nc = bacc.Bacc(None, target_bir_lowering=False)
    # xt is slot-major: block j is [P, KD*Cs[j]] at column KD*offs[j],
    # with xt[p, KD*offs[j] + k*Cs[j] + t] = x_slot_j[t, k*128 + p].
    # Slot 0's block alone gates the first matmul (~0.9 MB instead of
    # the whole 2.2 MB), so phase A starts ~4us sooner.
    xt = nc.dram_tensor("xt", [P, KD * CT], dt, kind="ExternalInput")
    # w1[e, c, p, k*512 + u] = w1[e][k*128 + p, c*512 + u]
    w1 = nc.dram_tensor("w1", [E_LOC, W1C, P, KD * W1CW], dt,
                        kind="ExternalInput")
    # w2[e, g, p, kk*1024 + c] = w2[e][(g*8 + kk)*128 + p, c]
    w2 = nc.dram_tensor("w2", [E_LOC, W2C, P, KPC * D_MODEL], dt,
                        kind="ExternalInput")
    b1 = nc.dram_tensor("b1", [P, E_LOC * KF], f32, kind="ExternalInput")
    yt = nc.dram_tensor("yt", [D_MODEL, CT], f32, kind="ExternalOutput")

    with TileContext(nc) as tc:
        with tc.tile_pool(name="consts", bufs=1) as consts, \
             tc.tile_pool(name="xtp", bufs=E_LOC) as xtp, \
             tc.tile_pool(name="w1p", bufs=W1C + 1) as w1p, \
             tc.tile_pool(name="w2p", bufs=W2C) as w2p, \
             tc.tile_pool(name="htp", bufs=KF + 2) as htp, \
             tc.tile_pool(name="outp", bufs=4) as outp, \
             tc.tile_pool(name="psA", bufs=4, space="PSUM") as psA, \
             tc.tile_pool(name="psB", bufs=3, space="PSUM") as psB:

            # slot 0's xt block first on the sync queue (startup critical
            # path is xt block 0 + w1 chunk 0 in queue order); the other
            # slots' blocks stream behind expert 0's w1.
            xts = []
            for j in range(E_LOC):
                xt_t = xtp.tile([P, KD * Cs[j]], dt, tag="xt")
                xts.append(xt_t)
            nc.sync.dma_start(xts[0][:],
                              xt[:, KD * offs[0]:KD * (offs[0] + Cs[0])])
            b1_t = consts.tile([P, E_LOC * KF], f32, tag="b1")
            nc.scalar.dma_start(b1_t[:], b1[:, :])

            # HAM warm-up: a fixed chain of dummy matmuls on a zeroed tile
            # spanning the startup DMA wait (~7us to ~14us), so the PE
            # clock gate is at full rate when the first real matmul issues.
            wu = consts.tile([P, 512], dt, tag="wu")
            nc.vector.memset(wu[:], 0)
            wups = psA.tile([P, 512], f32, tag="psA")
            for i in range(21):
                nc.tensor.matmul(wups[:], wu[:, :P], wu[:],
                                 start=True, stop=True)

            w1ts = {}  # chunk c -> [P, 4096] tile (current expert)
            w2ts = {}  # group g -> [P, 8192] tile (current expert)

            def load_w1(e):
                for c in range(W1C):
                    t_ = w1p.tile([P, KD * W1CW], dt, tag="w1")
                    nc.sync.dma_start(t_[:], w1[e, c, :, :])
                    w1ts[c] = t_

            def load_w2(e):
                # on the sync queue, after this expert's w1 in emission
                # order: the single-queue FIFO keeps the startup-critical
                # xt + w1 chunk 0 at full HBM bandwidth, and each w2
                # lands inside phase A of its expert.
                for g in range(W2C):
                    t_ = w2p.tile([P, KPC * D_MODEL], dt, tag="w2")
                    nc.sync.dma_start(t_[:], w2[e, g, :, :])
                    w2ts[g] = t_

            load_w1(0)
            for j in range(1, E_LOC):
                nc.sync.dma_start(xts[j][:],
                                  xt[:, KD * offs[j]:KD * (offs[j] + Cs[j])])
            for e in range(E_LOC):
                W = Cs[e]
                off = offs[e]
                load_w2(e)

                # phase A: hT[m] = gelu(w1[:,m].T @ xT + b1), [128 dff, W]
                hts = []
                for m in range(KF):
                    c = m // MPC
                    u0 = (m % MPC) * P
                    ps = psA.tile([P, 512], f32, tag="psA")
                    for k in range(KD):
                        nc.tensor.matmul(
                            ps[:, :W],
                            w1ts[c][:, k * W1CW + u0:k * W1CW + u0 + P],
                            xts[e][:, k * W:(k + 1) * W],
                            start=(k == 0), stop=(k == KD - 1))
                    ht = htp.tile([P, 512], dt, tag="ht")
                    nc.scalar.activation(
                        ht[:, :W], ps[:, :W],
                        mybir.ActivationFunctionType.Gelu,
                        bias=b1_t[:, e * KF + m: e * KF + m + 1])
                    hts.append(ht)

                # prefetch next expert's w1 during phase B
                if e + 1 < E_LOC:
                    load_w1(e + 1)

                # phase B: yT[m] = sum_k w2[k,m].T @ hT[k], [128 dm, W]
                for m in range(MD):
                    ps = psB.tile([P, 512], f32, tag="psB")
                    for k in range(KF):
                        g, kk = k // KPC, k % KPC
                        nc.tensor.matmul(
                            ps[:, :W],
                            w2ts[g][:, kk * D_MODEL + m * P:
                                    kk * D_MODEL + (m + 1) * P],
                            hts[k][:, :W],
                            start=(k == 0), stop=(k == KF - 1))
                    ot = outp.tile([P, 512], f32, tag="out")
                    nc.vector.tensor_scalar_mul(ot[:, :W], ps[:, :W], 1.0)
                    nc.scalar.dma_start(
                        yt[m * P:(m + 1) * P, off:off + W], ot[:, :W])
    nc.finalize()
    return nc


def _route(x, gate_w, gate_b):
    """Top-2 routing on host. Returns flattened (expert, prob) per routed
    pair, the by-expert sort order, per-expert counts/starts, and each
    pair's position within its expert segment."""
    T = x.shape[0]
    scores = x @ gate_w + gate_b                      # [T, E]
    part = np.argpartition(scores, -TOP_K, axis=1)[:, -TOP_K:]   # [T, 2]
    vals = np.take_along_axis(scores, part, axis=1)
    vmax = vals.max(axis=1, keepdims=True)
    ex = np.exp(vals - vmax)
    prob = ex / ex.sum(axis=1, keepdims=True)

    expert_flat = part.ravel()                        # [2T]
    prob_flat = prob.ravel().astype(np.float32)
    token_flat = np.repeat(np.arange(T), TOP_K)

    order = np.argsort(expert_flat, kind="stable")
    counts = np.bincount(expert_flat, minlength=NUM_EXPERTS)
    starts = np.zeros(NUM_EXPERTS + 1, dtype=np.int64)
    np.cumsum(counts, out=starts[1:])

    inv_order = np.empty_like(order)
    inv_order[order] = np.arange(order.size)
    pos = inv_order - starts[expert_flat]
    return (expert_flat, prob_flat, token_flat, order, counts, starts, pos)


def _prepare(x, gate_w, gate_b, w1, b1, w2, b2):
    """Host-side routing, balanced expert->(core,slot) assignment, and
    per-core input packing. Returns (in_maps, Cs, meta-for-combine)."""
    B, S, D = x.shape
    T = B * S
    xf = np.ascontiguousarray(x.reshape(T, D), dtype=np.float32)

    (expert_flat, prob_flat, token_flat, order, counts, starts, pos) = _route(
        xf, np.asarray(gate_w, np.float32), np.asarray(gate_b, np.float32))

    # balanced assignment: slot j of core c holds expert_desc[j*8 + c];
    # slot capacity = exact octile max (tokens are the free dim on device,
    # so no alignment is required).
    expert_desc = np.argsort(-counts, kind="stable")
    core_of = np.empty(NUM_EXPERTS, dtype=np.int64)
    slot_of = np.empty(NUM_EXPERTS, dtype=np.int64)
    for j in range(E_LOC):
        for c in range(N_CORES):
            e = expert_desc[j * N_CORES + c]
            core_of[e] = c
            slot_of[e] = j
    Cs = []
    for j in range(E_LOC):
        mx = counts[expert_desc[j * N_CORES:(j + 1) * N_CORES]].max()
        Cs.append(max(8, int(mx)))
    CT = sum(Cs)
    offs = [sum(Cs[:j]) for j in range(E_LOC)]

    xg16 = xf[token_flat[order]].astype(BF16)         # [2T, D] sorted by expert

    # weight re-layouts (same for every core; index per expert below)
    w1_16 = np.asarray(w1, np.float32).astype(BF16)   # [E, D, F]
    w2_16 = np.asarray(w2, np.float32).astype(BF16)   # [E, F, D]
    b1_f = np.asarray(b1, np.float32)                 # [E, F]
    # w1l[e, c, p, k*512+u] = w1[e, k*128+p, c*512+u]
    w1l = np.ascontiguousarray(
        w1_16.reshape(NUM_EXPERTS, KD, P, W1C, W1CW)
        .transpose(0, 3, 2, 1, 4)
        .reshape(NUM_EXPERTS, W1C, P, KD * W1CW))
    # w2l[e, g, p, kk*1024+c] = w2[e, (g*8+kk)*128+p, c]
    w2l = np.ascontiguousarray(
        w2_16.reshape(NUM_EXPERTS, W2C, KPC, P, D_MODEL)
        .transpose(0, 1, 3, 2, 4)
        .reshape(NUM_EXPERTS, W2C, P, KPC * D_MODEL))

    in_maps = []
    for c in range(N_CORES):
        # slot-major xt: block j is [P, KD, Cs[j]] at column KD*offs[j]
        xt_blocks = [np.zeros((P, KD, Cs[j]), dtype=BF16)
                     for j in range(E_LOC)]
        w1_core = np.empty((E_LOC, W1C, P, KD * W1CW), dtype=BF16)
        w2_core = np.empty((E_LOC, W2C, P, KPC * D_MODEL), dtype=BF16)
        b1_core = np.empty((E_LOC, D_FF), dtype=np.float32)
        for j in range(E_LOC):
            e = expert_desc[j * N_CORES + c]
            c_e = counts[e]
            if c_e:
                seg = slice(starts[e], starts[e] + c_e)
                # xg16[seg].T: [D, c_e] -> [KD, P, c_e] -> [P, KD, c_e]
                xt_blocks[j][:, :, :c_e] = (
                    xg16[seg].T.reshape(KD, P, c_e).transpose(1, 0, 2))
            w1_core[j] = w1l[e]
            w2_core[j] = w2l[e]
            b1_core[j] = b1_f[e]
        in_maps.append({
            "xt": np.ascontiguousarray(np.concatenate(
                [b.reshape(P, KD * Cs[j])
                 for j, b in enumerate(xt_blocks)], axis=1)),
            "w1": w1_core,
            "w2": w2_core,
            "b1": np.ascontiguousarray(
                b1_core.reshape(E_LOC, KF, P).transpose(2, 0, 1)
                .reshape(P, E_LOC * KF)),
        })

    meta = dict(T=T, shape=x.shape, CT=CT, offs=offs,
                core_of=core_of, slot_of=slot_of,
                expert_flat=expert_flat, prob_flat=prob_flat,
                token_flat=token_flat, pos=pos, b2=np.asarray(b2, np.float32))
    return in_maps, Cs, meta


def _combine(y_per_core, meta):
    """out[t] = sum over the token's two routed pairs of prob * yT[:, col]
    (+ b2 term). y_per_core[c] is [D_MODEL, CT] (transposed)."""
    T = meta["T"]
    CT = meta["CT"]
    offs = np.asarray(meta["offs"], dtype=np.int64)
    expert_flat = meta["expert_flat"]
    Y = np.concatenate(y_per_core, axis=1)            # [D, 8*CT]

    cols = (meta["core_of"][expert_flat] * CT
            + offs[meta["slot_of"][expert_flat]] + meta["pos"])
    cols = cols.reshape(T, TOP_K)
    pr = meta["prob_flat"].reshape(T, TOP_K)
    out = (Y[:, cols[:, 0]] * pr[:, 0] + Y[:, cols[:, 1]] * pr[:, 1]).T

    b2_f = meta["b2"]
    if np.any(b2_f):
        combine = np.zeros((T, NUM_EXPERTS), dtype=np.float32)
        np.add.at(combine, (meta["token_flat"], expert_flat), meta["prob_flat"])
        out = out + combine @ b2_f
    return np.ascontiguousarray(out.reshape(meta["shape"]), dtype=np.float32)


def kernel(x, gate_w, gate_b, w1, b1, w2, b2):
    from concourse import bass_utils

    in_maps, Cs, meta = _prepare(x, gate_w, gate_b, w1, b1, w2, b2)
    nc = _build(Cs)
    res = bass_utils.run_bass_kernel_spmd(nc, in_maps, core_ids=list(range(N_CORES)))
    return _combine([res.results[c]["yt"] for c in range(N_CORES)], meta)
